# revision 1
# baseline (speedup 1.0000x reference)
"""GroupedQueryAttention Trainium2 kernel.

B=2, S=2048, D_MODEL=2048, 32 query heads / 8 KV heads, d_k=64.
Sharding: 8 cores = 2 (batch) x 4 (head groups of 8 query heads / 2 KV heads).
Per core: Wq/Wk/Wv column shard, Wo row shard; host sums the 4 partial
outputs per batch (the "all-reduce" of the row-parallel output projection).

Per-core device schedule (everything float32r on the PE at full rate):
  phase 1: Q^T, K^T, V^T projections from host-transposed x^T; V^T is
           PE-transposed back to natural [token, dim] layout and augmented
           with a ones column (softmax denominator rides the ctx matmul).
  phase 2: per head-pair, per 512-query tile: scores^T = K_T.T @ Q_T with the
           two heads row-packed into PE strips (rows 0-63 / 64-127, via a
           partition-duplicated K^T), exp on ScalarE straight out of PSUM
           (scale=1/8 folded into the activation), ctx^T accumulated as
           V_aug.T @ expS^T (m=65: 64 ctx dims + denominator row),
           normalization folded into the PSUM eviction.
  phase 3: partial output projection ctx^T.T @ Wo_rows -> DMA out.
"""

import sys

sys.path.insert(0, "/opt/trn_rl_repo")

import numpy as np

import concourse.bass as bass
import concourse.tile as tile
from concourse import bacc, mybir
from concourse.bass_utils import run_bass_kernel_spmd
from concourse.masks import make_identity

F32 = mybir.dt.float32
F32R = mybir.dt.float32r
F16 = mybir.dt.float16

D = 2048          # d_model
S = 2048          # sequence length
HL = 8            # query heads per core
KVL = 2           # kv heads per core
DK = 64
QO = HL * DK      # 512 query outdims per core
KO = KVL * DK     # 128 kv outdims per core
NKT = 16          # d_model contraction tiles of 128
NTT = 16          # token tiles of 128
NQT = 4           # query tiles of 512
EG = 2            # key-tiles per exp group

_CACHE = {}


def _build_nc():
    nc = bacc.Bacc("TRN2", target_bir_lowering=False)

    xT_h = nc.dram_tensor("xT", [D, S], F16, kind="ExternalInput")
    wq_h = nc.dram_tensor("wq", [D, QO], F16, kind="ExternalInput")
    wk_h = nc.dram_tensor("wk", [D, KO], F16, kind="ExternalInput")
    wv_h = nc.dram_tensor("wv", [D, KO], F16, kind="ExternalInput")
    wo_h = nc.dram_tensor("wo", [QO, D], F16, kind="ExternalInput")
    bq_h = nc.dram_tensor("bq2", [128, 4], F32, kind="ExternalInput")
    bk_h = nc.dram_tensor("bk2", [128, 1], F32, kind="ExternalInput")
    bv_h = nc.dram_tensor("bv2", [128, 1], F32, kind="ExternalInput")
    out_h = nc.dram_tensor("out", [S, D], F32, kind="ExternalOutput")

    with tile.TileContext(nc) as tc:
        _emit(nc, tc, xT_h, wq_h, wk_h, wv_h, wo_h, bq_h, bk_h, bv_h, out_h)
    nc.compile()
    return nc


def _emit(nc, tc, xT_h, wq_h, wk_h, wv_h, wo_h, bq_h, bk_h, bv_h, out_h):
    from contextlib import ExitStack

    ctx = ExitStack()
    with ctx:
        consts = ctx.enter_context(tc.tile_pool(name="consts", bufs=1))
        projout = ctx.enter_context(tc.tile_pool(name="projout", bufs=1))
        mmps = ctx.enter_context(tc.tile_pool(name="mmps", bufs=3, space="PSUM"))
        accps = ctx.enter_context(tc.tile_pool(name="accps", bufs=1, space="PSUM"))

        ident = consts.tile([128, 128], F16)
        make_identity(nc, ident)

        # persistent projection outputs
        qt_sb = projout.tile([128, 4, S], F16)    # [dim-in-pair, pair, token]
        ktd_sb = projout.tile([128, KVL, S], F16)  # kv dims duplicated both halves
        vt_sb = projout.tile([128, S], F16)        # [kv dims (2x64), token]
        vaug_sb = projout.tile([128, NTT, KVL, 128], F16)  # [tok, tok-tile, kv, dim|ones]
        ctxT_sb = projout.tile([128, 4, S], F16)   # [dim-in-pair, pair, token]

        bq_sb = consts.tile([128, 4], F32)
        bk_sb = consts.tile([128, 1], F32)
        bv_sb = consts.tile([128, 1], F32)
        nc.sync.dma_start(out=bq_sb, in_=bq_h[:])
        nc.sync.dma_start(out=bk_sb, in_=bk_h[:])
        nc.sync.dma_start(out=bv_sb, in_=bv_h[:])

        # ---------------- phase 1: projections ----------------
        with tc.tile_pool(name="xt", bufs=4) as xtp, \
             tc.tile_pool(name="wqkv", bufs=1) as wp:
            wq_sb = wp.tile([128, NKT, QO], F16)
            wk_sb = wp.tile([128, NKT, KO], F16)
            wv_sb = wp.tile([128, NKT, KO], F16)
            nc.sync.dma_start(out=wq_sb, in_=wq_h.rearrange("(k p) m -> p k m", p=128))
            nc.sync.dma_start(out=wk_sb, in_=wk_h.rearrange("(k p) m -> p k m", p=128))
            nc.sync.dma_start(out=wv_sb, in_=wv_h.rearrange("(k p) m -> p k m", p=128))

            xT_r = xT_h.rearrange("(k p) t -> p k t", p=128)
            xts = []
            for nt in range(4):  # 512-token slabs, all held resident
                ns = slice(nt * 512, (nt + 1) * 512)
                xt_t = xtp.tile([128, NKT, 512], F16, tag="xt", name=f"xt{nt}")
                nc.sync.dma_start(out=xt_t, in_=xT_r[:, :, ns])
                xts.append(xt_t)

            # K then V first (attention needs them for every query tile),
            # Q last so attention can start while late Q slabs project.
            for nt in range(4):
                ns = slice(nt * 512, (nt + 1) * 512)
                ps = mmps.tile([128, EG, 512], F32)
                for kt in range(NKT):
                    nc.tensor.matmul(
                        ps[:, 0, :], lhsT=wk_sb[:, kt, :], rhs=xts[nt][:, kt, :],
                        start=(kt == 0), stop=(kt == NKT - 1))
                nc.vector.tensor_scalar_add(ps[:, 1, :], ps[:, 0, :],
                                            bk_sb[:, 0:1])
                for kv in range(KVL):
                    src = ps[kv * 64:(kv + 1) * 64, 1, 0:512]
                    nc.vector.tensor_copy(ktd_sb[0:64, kv, ns], src)
                    nc.vector.tensor_copy(ktd_sb[64:128, kv, ns], src)

            for nt in range(4):
                ns = slice(nt * 512, (nt + 1) * 512)
                ps = mmps.tile([128, EG, 512], F32)
                for kt in range(NKT):
                    nc.tensor.matmul(
                        ps[:, 0, :], lhsT=wv_sb[:, kt, :], rhs=xts[nt][:, kt, :],
                        start=(kt == 0), stop=(kt == NKT - 1))
                nc.vector.tensor_scalar_add(vt_sb[:, ns], ps[:, 0, :], bv_sb[:, 0:1])

            for nt in range(4):
                ns = slice(nt * 512, (nt + 1) * 512)
                for mt in range(4):  # Q^T m-tiles (= head pairs)
                    ps = mmps.tile([128, EG, 512], F32)
                    for kt in range(NKT):
                        nc.tensor.matmul(
                            ps[:, 0, :],
                            lhsT=wq_sb[:, kt, mt * 128:(mt + 1) * 128],
                            rhs=xts[nt][:, kt, :],
                            start=(kt == 0), stop=(kt == NKT - 1),
                        )
                    nc.vector.tensor_scalar_add(
                        qt_sb[:, mt, ns], ps[:, 0, :], bq_sb[:, mt:mt + 1])

        # V^T -> natural V layout via PE transpose, build V_aug with ones col
        for tt in range(NTT):
            pst = mmps.tile([128, EG, 512], F16, tag="ps", name="pst")
            nc.tensor.transpose(
                pst[:, 0, 0:128],
                vt_sb[:, tt * 128:(tt + 1) * 128],
                ident[:],
            )
            for kv in range(KVL):
                nc.vector.tensor_copy(
                    vaug_sb[:, tt, kv, 0:64], pst[:, 0, kv * 64:(kv + 1) * 64])
        ones_sb = consts.tile([128, 64], F16)
        nc.vector.memset(ones_sb, 1.0)
        for tt in range(NTT):
            for kv in range(KVL):
                nc.vector.tensor_copy(vaug_sb[:, tt, kv, 64:128], ones_sb)

        # ---------------- phase 2: attention ----------------
        with tc.tile_pool(name="expst", bufs=6) as ep, \
             tc.tile_pool(name="rden", bufs=4) as rp, \
             tc.tile_pool(name="wo", bufs=1) as wop:
            wo_sb = wop.tile([128, 4, D], F16)
            nc.sync.dma_start(out=wo_sb, in_=wo_h.rearrange("(c p) d -> p c d", p=128))

            for qt in range(NQT):
                qs = slice(qt * 512, (qt + 1) * 512)
                for pair in range(4):
                    kv = pair // 2
                    ctx_ps = [accps.tile([128, 512], F32, tag=f"ctx{i}", name=f"ctx{i}") for i in range(2)]
                    for g in range(NTT // EG):
                        sp = [mmps.tile([128, EG, 512], F32, tag="ps", name=f"sp{i}") for i in range(2)]
                        for j in range(EG):
                            ktile = g * EG + j
                            ks = slice(ktile * 128, (ktile + 1) * 128)
                            for i in range(2):  # head i of the pair
                                nc.tensor.matmul(
                                    sp[i][:, j, :],
                                    lhsT=ktd_sb[i * 64:(i + 1) * 64, kv, ks],
                                    rhs=qt_sb[i * 64:(i + 1) * 64, pair, qs],
                                    start=True, stop=True,
                                    tile_position=(i * 64, 0),
                                )
                        es = []
                        for i in range(2):
                            e = ep.tile([128, EG, 512], F16)
                            nc.scalar.activation(
                                e[:, :, :], sp[i][:, :, :],
                                mybir.ActivationFunctionType.Exp, scale=0.125)
                            es.append(e)
                        for j in range(EG):
                            ktile = g * EG + j
                            for i in range(2):
                                nc.tensor.matmul(
                                    ctx_ps[i][:, :],
                                    lhsT=vaug_sb[:, ktile, kv, :],
                                    rhs=es[i][:, j, :],
                                    start=(ktile == 0), stop=(ktile == NTT - 1),
                                )
                    for i in range(2):
                        rdb = rp.tile([64, 512], F32)
                        nc.vector.reciprocal(rdb, ctx_ps[i][64:128, :])
                        nc.vector.tensor_tensor(
                            ctxT_sb[i * 64:(i + 1) * 64, pair, qs],
                            ctx_ps[i][0:64, :],
                            rdb[:, :],
                            mybir.AluOpType.mult,
                        )

            # ---------------- phase 3: output projection ----------------
            with tc.tile_pool(name="osb", bufs=4) as op:
                for tt in range(NTT):
                    ts_ = slice(tt * 128, (tt + 1) * 128)
                    for dn in range(4):
                        ds_ = slice(dn * 512, (dn + 1) * 512)
                        ps = mmps.tile([128, EG, 512], F32)
                        for c in range(4):
                            nc.tensor.matmul(
                                ps[:, 0, :],
                                lhsT=ctxT_sb[:, c, ts_],
                                rhs=wo_sb[:, c, ds_],
                                start=(c == 0), stop=(c == 3),
                            )
                        ob = op.tile([128, 512], F32)
                        nc.vector.tensor_copy(ob, ps[:, 0, :])
                        nc.sync.dma_start(out=out_h[ts_, ds_], in_=ob)


def _get_nc():
    if "nc" not in _CACHE:
        _CACHE["nc"] = _build_nc()
    return _CACHE["nc"]


def kernel(x, Wq, bq, Wk, bk, Wv, bv, Wo, bo, _trace=False):
    x = np.asarray(x, np.float32)
    Wq = np.asarray(Wq, np.float32)
    bq = np.asarray(bq, np.float32)
    Wk = np.asarray(Wk, np.float32)
    bk = np.asarray(bk, np.float32)
    Wv = np.asarray(Wv, np.float32)
    bv = np.asarray(bv, np.float32)
    Wo = np.asarray(Wo, np.float32)
    bo = np.asarray(bo, np.float32)

    nc = _get_nc()
    in_maps = []
    for r in range(8):
        b, g = divmod(r, 4)
        qsl = slice(g * 512, (g + 1) * 512)
        ksl = slice(g * 128, (g + 1) * 128)
        in_maps.append({
            "xT": np.ascontiguousarray(x[b].T.astype(np.float16)),
            "wq": np.ascontiguousarray(Wq[:, qsl].astype(np.float16)),
            "wk": np.ascontiguousarray(Wk[:, ksl].astype(np.float16)),
            "wv": np.ascontiguousarray(Wv[:, ksl].astype(np.float16)),
            "wo": np.ascontiguousarray(Wo[qsl, :].astype(np.float16)),
            "bq2": np.ascontiguousarray(bq[qsl].reshape(4, 128).T),
            "bk2": np.ascontiguousarray(bk[ksl].reshape(128, 1)),
            "bv2": np.ascontiguousarray(bv[ksl].reshape(128, 1)),
        })

    res = run_bass_kernel_spmd(nc, in_maps, list(range(8)), trace=_trace)
    out = np.zeros((2, S, D), np.float64)
    for r in range(8):
        out[r // 4] += res.results[r]["out"].astype(np.float64)
    out += bo.astype(np.float64)
    result = out.astype(np.float32)
    if _trace:
        return result, res
    return result



# revision 6
# speedup vs baseline: 1.0043x; 1.0043x over previous
"""GroupedQueryAttention Trainium2 kernel (pipelined).

B=2, S=2048, D_MODEL=2048, 32 query heads / 8 KV heads, d_k=64.
Sharding: 8 cores = 2 (batch) x 4 (head groups of 8 query heads / 2 KV heads).
Per core: Wq/Wk/Wv column shard, Wo row shard; host sums the 4 partial
outputs per batch (the "all-reduce" of the row-parallel output projection).

Per-core schedule: one software-pipelined stream. ScalarE (exp) is the
critical engine (256 x 1024-elem EXPs ~= 294us busy); everything else is
arranged around keeping it fed:
  - attention on query-tile 0 / pair 0 starts right after K(slab0),
    Q(qt0,pair0) and V(slab0) project (~17us in)
  - all other projections and the output projection of finished query
    tiles are "aux" work interleaved into the PE queue between score
    groups, scheduled against emission-order deadlines (the PE queue is
    in-order, so every producer must be emitted before its consumer)
  - V transpose to natural layout rides the DMA xbar engine
    (dma_start_transpose), not the PE
  - softmax denominator rides the ctx matmul (ones half of vaug);
    normalization: one FD-1024 PSUM->SBUF copy (frees the ctx bank in
    ~1.2us), then reciprocal_approx_fast + 2 multiplies off the copy
  - PSUM: 2x2-bank score tiles + 1x2-bank ctx + 1-bank one-shot pool
    (ph3) + 1-bank accumulation pool (K/V/Q proj) = 8 banks exactly
Output is written f16 (host upcasts and sums partials in f64).
"""

import sys

sys.path.insert(0, "/opt/trn_rl_repo")

import numpy as np

import concourse.bass as bass
import concourse.tile as tile
from concourse import bacc, mybir
from concourse.bass_utils import run_bass_kernel_spmd

F32 = mybir.dt.float32
F16 = mybir.dt.float16

D = 2048          # d_model
S = 2048          # sequence length
HL = 8            # query heads per core
KVL = 2           # kv heads per core
DK = 64
QO = HL * DK      # 512 query outdims per core
KO = KVL * DK     # 128 kv outdims per core
NKT = 16          # d_model contraction tiles of 128
NTT = 16          # token tiles of 128
NQT = 4           # query tiles of 512
NSLOT = 32        # score groups per query tile (4 pairs x 8 groups)

_CACHE = {}


def _build_nc():
    nc = bacc.Bacc("TRN2", target_bir_lowering=False)

    xT_h = nc.dram_tensor("xT", [D, S], F16, kind="ExternalInput")
    wq_h = nc.dram_tensor("wq", [D, QO], F16, kind="ExternalInput")
    wk_h = nc.dram_tensor("wk", [D, KO], F16, kind="ExternalInput")
    wv_h = nc.dram_tensor("wv", [D, KO], F16, kind="ExternalInput")
    wo_h = nc.dram_tensor("wo", [QO, D], F16, kind="ExternalInput")
    bq_h = nc.dram_tensor("bq2", [128, 4], F32, kind="ExternalInput")
    bk_h = nc.dram_tensor("bk2", [128, 1], F32, kind="ExternalInput")
    bv_h = nc.dram_tensor("bv2", [128, 1], F32, kind="ExternalInput")
    out_h = nc.dram_tensor("out", [S, D], F16, kind="ExternalOutput")

    with tile.TileContext(nc) as tc:
        _emit(nc, tc, xT_h, wq_h, wk_h, wv_h, wo_h, bq_h, bk_h, bv_h, out_h)
    nc.compile()
    return nc


def _emit(nc, tc, xT_h, wq_h, wk_h, wv_h, wo_h, bq_h, bk_h, bv_h, out_h):
    from contextlib import ExitStack

    ctx = ExitStack()
    with ctx:
        consts = ctx.enter_context(tc.tile_pool(name="consts", bufs=1))
        persist = ctx.enter_context(tc.tile_pool(name="persist", bufs=1))
        xtp = ctx.enter_context(tc.tile_pool(name="xt", bufs=4))
        ep = ctx.enter_context(tc.tile_pool(name="expst", bufs=12))
        vnp = ctx.enter_context(tc.tile_pool(name="vnat", bufs=4))
        rawp = ctx.enter_context(tc.tile_pool(name="ctxraw", bufs=2))
        rdbp = ctx.enter_context(tc.tile_pool(name="rdb", bufs=2))
        obp = ctx.enter_context(tc.tile_pool(name="osb", bufs=4))
        spps = ctx.enter_context(tc.tile_pool(name="spps", bufs=2, space="PSUM"))
        ctxps = ctx.enter_context(tc.tile_pool(name="ctxps", bufs=1, space="PSUM"))
        # one-shot PSUM users (ph3 chunks): released after one eviction
        shotps = ctx.enter_context(tc.tile_pool(name="shotps", bufs=1, space="PSUM"))
        # multi-slot accumulations (K/V/Q proj), held across interleaved
        # aux items -- must not share a pool with one-shot users
        accps = ctx.enter_context(tc.tile_pool(name="accps", bufs=1, space="PSUM"))

        # persistent SBUF tensors
        qt_sb = persist.tile([128, 4, S], F16)      # Q^T [dim-in-pair, pair, token]
        ktd_sb = persist.tile([128, KVL, S], F16)   # K^T, kv dims dup both halves
        vt_sb = persist.tile([128, S], F16)         # V^T [kv dims (2x64), token]
        vaug_sb = persist.tile([128, NTT, KVL, 128], F16)  # [tok, tt, kv, dim|ones]
        ctxT_sb = persist.tile([128, 4, S], F16)    # [dim-in-pair, pair, token]

        wq_sb = persist.tile([128, NKT, QO], F16)
        wk_sb = persist.tile([128, NKT, KO], F16)
        wv_sb = persist.tile([128, NKT, KO], F16)
        wo_sb = persist.tile([128, 4, D], F16)

        bq_sb = consts.tile([128, 4], F32)
        bk_sb = consts.tile([128, 1], F32)
        bv_sb = consts.tile([128, 1], F32)

        # ---- input DMAs, ordered by first use ----
        nc.sync.dma_start(out=bq_sb, in_=bq_h[:])
        nc.sync.dma_start(out=bk_sb, in_=bk_h[:])
        nc.sync.dma_start(out=bv_sb, in_=bv_h[:])
        nc.sync.dma_start(out=wk_sb, in_=wk_h.rearrange("(k p) m -> p k m", p=128))
        nc.sync.dma_start(out=wv_sb, in_=wv_h.rearrange("(k p) m -> p k m", p=128))

        xT_r = xT_h.rearrange("(k p) t -> p k t", p=128)
        wq_r = wq_h.rearrange("(k p) m -> p k m", p=128)
        xts = []
        for nt in range(4):
            xts.append(xtp.tile([128, NKT, 512], F16, tag="xt", name=f"xt{nt}"))
        nc.sync.dma_start(out=xts[0], in_=xT_r[:, :, 0:512])
        # pair-0 query weights first so attention can start early
        nc.sync.dma_start(out=wq_sb[:, :, 0:128], in_=wq_r[:, :, 0:128])
        nc.sync.dma_start(out=xts[1], in_=xT_r[:, :, 512:1024])
        nc.sync.dma_start(out=xts[2], in_=xT_r[:, :, 1024:1536])
        nc.sync.dma_start(out=xts[3], in_=xT_r[:, :, 1536:2048])
        nc.sync.dma_start(out=wq_sb[:, :, 128:512], in_=wq_r[:, :, 128:512])
        nc.sync.dma_start(out=wo_sb, in_=wo_h.rearrange("(c p) d -> p c d", p=128))

        ones_init = [False]

        # ---------- aux work item generators (each returns a closure) ----------
        kproj_state = {}

        def kproj_piece(nt, piece):
            def run():
                ns = slice(nt * 512, (nt + 1) * 512)
                if piece == 0:
                    kproj_state[nt] = accps.tile([128, 512], F32, tag="acc", name=f"kp{nt}")
                ps = kproj_state[nt]
                for kt in range(piece * 4, piece * 4 + 4):
                    nc.tensor.matmul(
                        ps[:, :], lhsT=wk_sb[:, kt, :], rhs=xts[nt][:, kt, :],
                        start=(kt == 0), stop=(kt == NKT - 1))
                if piece == 3:
                    # bias + cast + duplicate kv dims into both partition halves
                    for kv in range(KVL):
                        src = ps[kv * 64:(kv + 1) * 64, 0:512]
                        sc = bk_sb[kv * 64:(kv + 1) * 64, 0:1]
                        nc.vector.tensor_scalar_add(ktd_sb[0:64, kv, ns], src, sc)
                        nc.vector.tensor_scalar_add(ktd_sb[64:128, kv, ns], src, sc)
            return run

        vproj_state = {}
        vnat_state = {}

        def vproj_piece(nt, piece):
            def run():
                ns = slice(nt * 512, (nt + 1) * 512)
                if piece == 0:
                    vproj_state[nt] = accps.tile([128, 512], F32, tag="acc", name=f"vp{nt}")
                ps = vproj_state[nt]
                for kt in range(piece * 4, piece * 4 + 4):
                    nc.tensor.matmul(
                        ps[:, :], lhsT=wv_sb[:, kt, :], rhs=xts[nt][:, kt, :],
                        start=(kt == 0), stop=(kt == NKT - 1))
                if piece == 3:
                    nc.vector.tensor_scalar_add(vt_sb[:, ns], ps[:, :],
                                                bv_sb[:, 0:1])
                    # V -> natural [token, dim] layout via DMA xbar transpose
                    for tt in range(nt * 4, nt * 4 + 4):
                        vn = vnp.tile([128, 128], F16, tag="vn", name=f"vn{tt}")
                        vnat_state[tt] = vn
                        nc.sync.dma_start_transpose(
                            vn, vt_sb[:, tt * 128:(tt + 1) * 128])
            return run

        def transp_copies(tt):
            # DVE-only: vnat slices into vaug (ones half pre-set by memset)
            def run():
                if not ones_init[0]:
                    nc.vector.memset(vaug_sb, 1.0)
                    ones_init[0] = True
                vn = vnat_state[tt]
                for kv in range(KVL):
                    nc.vector.tensor_copy(
                        vaug_sb[:, tt, kv, 0:64],
                        vn[:, kv * 64:(kv + 1) * 64])
            return run

        qproj_state = {}

        def qproj_piece(qt, mt, piece):
            def run():
                ns = slice(qt * 512, (qt + 1) * 512)
                if piece == 0:
                    qproj_state[(qt, mt)] = accps.tile([128, 512], F32, tag="acc", name=f"qp{qt}_{mt}")
                ps = qproj_state[(qt, mt)]
                for kt in range(piece * 4, piece * 4 + 4):
                    nc.tensor.matmul(
                        ps[:, :],
                        lhsT=wq_sb[:, kt, mt * 128:(mt + 1) * 128],
                        rhs=xts[qt][:, kt, :],
                        start=(kt == 0), stop=(kt == NKT - 1))
                if piece == 3:
                    nc.vector.tensor_scalar_add(
                        qt_sb[:, mt, ns], ps[:, :], bq_sb[:, mt:mt + 1])
            return run

        def ph3_chunk(qt, tt, dn, alt=False):
            def run():
                ts_ = slice(tt * 128, (tt + 1) * 128)
                ds_ = slice(dn * 512, (dn + 1) * 512)
                if alt:
                    ps = accps.tile([128, 512], F32, tag="acc", name=f"p3a{tt}_{dn}")
                else:
                    ps = shotps.tile([128, 512], F32, tag="shot", name=f"p3s{tt}_{dn}")
                for c in range(4):
                    nc.tensor.matmul(
                        ps[:, :],
                        lhsT=ctxT_sb[:, c, ts_],
                        rhs=wo_sb[:, c, ds_],
                        start=(c == 0), stop=(c == 3),
                    )
                ob = obp.tile([128, 512], F16)
                nc.vector.tensor_copy(ob, ps)
                nc.sync.dma_start(out=out_h[ts_, ds_], in_=ob)
            return run

        # ---------- pre-attention head ----------
        for piece in range(4):
            kproj_piece(0, piece)()
        for piece in range(4):
            qproj_piece(0, 0, piece)()
        for piece in range(4):
            vproj_piece(0, piece)()
        for tt in range(4):
            transp_copies(tt)()

        # ---------- aux slot plans (per qt: NSLOT slots) ----------
        def grp(f, *idx_lists):
            return [f(*i) for i in idx_lists]

        slots = {qt: [[] for _ in range(NSLOT)] for qt in range(NQT)}
        s0 = slots[0]
        s0[0] = [kproj_piece(1, p) for p in range(4)]
        s0[1] = [vproj_piece(1, p) for p in range(4)] + \
                [transp_copies(t) for t in range(4, 8)]
        s0[2] = [kproj_piece(2, p) for p in range(4)]
        s0[3] = [vproj_piece(2, p) for p in range(4)] + \
                [transp_copies(t) for t in range(8, 12)]
        s0[4] = [kproj_piece(3, p) for p in range(4)]
        s0[5] = [vproj_piece(3, p) for p in range(4)] + \
                [transp_copies(t) for t in range(12, 16)]
        s0[6] = [qproj_piece(0, 1, p) for p in range(4)]
        s0[8] = [qproj_piece(0, 2, p) for p in range(4)]
        s0[10] = [qproj_piece(0, 3, p) for p in range(4)]
        s0[12] = [qproj_piece(1, 0, p) for p in range(2)]
        s0[13] = [qproj_piece(1, 0, p) for p in range(2, 4)]
        s0[15] = [qproj_piece(1, 1, p) for p in range(2)]
        s0[16] = [qproj_piece(1, 1, p) for p in range(2, 4)]
        s0[18] = [qproj_piece(1, 2, p) for p in range(2)]
        s0[19] = [qproj_piece(1, 2, p) for p in range(2, 4)]
        s0[21] = [qproj_piece(1, 3, p) for p in range(2)]
        s0[22] = [qproj_piece(1, 3, p) for p in range(2, 4)]

        for qt in (1, 2):
            sq = slots[qt]
            # output projection of qt-1: one chunk per slot
            items = [(tt, dn) for tt in range((qt - 1) * 4, (qt - 1) * 4 + 4)
                     for dn in range(4)]
            for k, (tt, dn) in enumerate(items):
                sq[k] = [ph3_chunk(qt - 1, tt, dn)]
            # Q projection of qt+1: 2 pieces per slot
            base = 17
            for mt in range(4):
                sq[base + 3 * mt] = [qproj_piece(qt + 1, mt, p) for p in range(2)]
                sq[base + 3 * mt + 1] = [qproj_piece(qt + 1, mt, p)
                                         for p in range(2, 4)]
        sq = slots[3]
        items = [(tt, dn) for tt in range(8, 12) for dn in range(4)]
        for k, (tt, dn) in enumerate(items):
            sq[k] = [ph3_chunk(2, tt, dn)]

        # ---------- main attention loop ----------
        for qt in range(NQT):
            qs = slice(qt * 512, (qt + 1) * 512)
            for pair in range(4):
                kv = pair // 2
                ctx_t = ctxps.tile([128, 2, 512], F32, tag="ctx")
                for g in range(8):
                    sp = [spps.tile([128, 2, 512], F32, tag="sp", name=f"sp{i}")
                          for i in range(2)]
                    for j in range(2):
                        kt = g * 2 + j
                        ks = slice(kt * 128, (kt + 1) * 128)
                        for i in range(2):
                            nc.tensor.matmul(
                                sp[i][:, j, :],
                                lhsT=ktd_sb[i * 64:(i + 1) * 64, kv, ks],
                                rhs=qt_sb[i * 64:(i + 1) * 64, pair, qs],
                                start=True, stop=True,
                                tile_position=(i * 64, 0),
                            )
                    es = []
                    for i in range(2):
                        e = ep.tile([128, 2, 512], F16)
                        nc.scalar.activation(
                            e[:, :, :], sp[i][:, :, :],
                            mybir.ActivationFunctionType.Exp, scale=0.125)
                        es.append(e)
                    for j in range(2):
                        kt = g * 2 + j
                        for i in range(2):
                            nc.tensor.matmul(
                                ctx_t[:, i, :],
                                lhsT=vaug_sb[:, kt, kv, :],
                                rhs=es[i][:, j, :],
                                start=(kt == 0), stop=(kt == NTT - 1),
                            )
                    for item in slots[qt][pair * 8 + g]:
                        item()

                # evict ctx bank fast, then normalize off the raw copy
                raw = rawp.tile([128, 2, 512], F32, tag="raw")
                nc.vector.tensor_copy(raw, ctx_t)
                rdb = rdbp.tile([64, 2, 512], F32, tag="rdb")
                nc.vector.reciprocal(rdb, raw[64:128, :, :])
                for i in range(2):
                    nc.vector.tensor_tensor(
                        ctxT_sb[i * 64:(i + 1) * 64, pair, qs],
                        raw[0:64, i, :],
                        rdb[:, i, :],
                        mybir.AluOpType.mult,
                    )

        # ---------- tail: output projection of qt3 (alternate psum pools) ----
        tail = [(tt, dn) for tt in range(12, 16) for dn in range(4)]
        for n, (tt, dn) in enumerate(tail):
            ph3_chunk(3, tt, dn, alt=(n % 2 == 1))()


def _get_nc():
    if "nc" not in _CACHE:
        _CACHE["nc"] = _build_nc()
    return _CACHE["nc"]


def kernel(x, Wq, bq, Wk, bk, Wv, bv, Wo, bo, _trace=False):
    x = np.asarray(x, np.float32)
    Wq = np.asarray(Wq, np.float32)
    bq = np.asarray(bq, np.float32)
    Wk = np.asarray(Wk, np.float32)
    bk = np.asarray(bk, np.float32)
    Wv = np.asarray(Wv, np.float32)
    bv = np.asarray(bv, np.float32)
    Wo = np.asarray(Wo, np.float32)
    bo = np.asarray(bo, np.float32)

    nc = _get_nc()
    in_maps = []
    for r in range(8):
        b, g = divmod(r, 4)
        qsl = slice(g * 512, (g + 1) * 512)
        ksl = slice(g * 128, (g + 1) * 128)
        in_maps.append({
            "xT": np.ascontiguousarray(x[b].T.astype(np.float16)),
            "wq": np.ascontiguousarray(Wq[:, qsl].astype(np.float16)),
            "wk": np.ascontiguousarray(Wk[:, ksl].astype(np.float16)),
            "wv": np.ascontiguousarray(Wv[:, ksl].astype(np.float16)),
            "wo": np.ascontiguousarray(Wo[qsl, :].astype(np.float16)),
            "bq2": np.ascontiguousarray(bq[qsl].reshape(4, 128).T),
            "bk2": np.ascontiguousarray(bk[ksl].reshape(128, 1)),
            "bv2": np.ascontiguousarray(bv[ksl].reshape(128, 1)),
        })

    res = run_bass_kernel_spmd(nc, in_maps, list(range(8)), trace=_trace)
    out = np.zeros((2, S, D), np.float64)
    for r in range(8):
        out[r // 4] += res.results[r]["out"].astype(np.float64)
    out += bo.astype(np.float64)
    result = out.astype(np.float32)
    if _trace:
        return result, res
    return result


# revision 7
# speedup vs baseline: 1.0395x; 1.0350x over previous
"""GroupedQueryAttention Trainium2 kernel (pipelined).

B=2, S=2048, D_MODEL=2048, 32 query heads / 8 KV heads, d_k=64.
Sharding: 8 cores = 2 (batch) x 4 (head groups of 8 query heads / 2 KV heads).
Per core: Wq/Wk/Wv column shard, Wo row shard; host sums the 4 partial
outputs per batch (the "all-reduce" of the row-parallel output projection).

Per-core schedule: one software-pipelined stream. ScalarE (exp) is the
critical engine (256 x 1024-elem EXPs ~= 294us busy); everything else is
arranged around keeping it fed:
  - attention on query-tile 0 / pair 0 starts right after K(slab0),
    Q(qt0,pair0) and V(slab0) project (~17us in)
  - all other projections and the output projection of finished query
    tiles are "aux" work interleaved into the PE queue between score
    groups, scheduled against emission-order deadlines (the PE queue is
    in-order, so every producer must be emitted before its consumer)
  - V transpose to natural layout rides the DMA xbar engine
    (dma_start_transpose), not the PE
  - softmax denominator rides the ctx matmul (ones half of vaug);
    normalization: one FD-1024 PSUM->SBUF copy (frees the ctx bank in
    ~1.2us), then reciprocal_approx_fast + 2 multiplies off the copy
  - PSUM: 2x2-bank score tiles + 1x2-bank ctx + 1-bank one-shot pool
    (ph3) + 1-bank accumulation pool (K/V/Q proj) = 8 banks exactly
Output is written f16 (host upcasts and sums partials in f64).
"""

import sys

sys.path.insert(0, "/opt/trn_rl_repo")

import numpy as np

import concourse.bass as bass
import concourse.tile as tile
from concourse import bacc, mybir
from concourse.bass_utils import run_bass_kernel_spmd

F32 = mybir.dt.float32
F16 = mybir.dt.float16

D = 2048          # d_model
S = 2048          # sequence length
HL = 8            # query heads per core
KVL = 2           # kv heads per core
DK = 64
QO = HL * DK      # 512 query outdims per core
KO = KVL * DK     # 128 kv outdims per core
NKT = 16          # d_model contraction tiles of 128
NTT = 16          # token tiles of 128
NQT = 4           # query tiles of 512
NSLOT = 32        # score groups per query tile (4 pairs x 8 groups)

_CACHE = {}


def _build_nc():
    nc = bacc.Bacc("TRN2", target_bir_lowering=False)

    xT_h = nc.dram_tensor("xT", [128, 4, NKT, 512], F16, kind="ExternalInput")
    wq_h = nc.dram_tensor("wq", [128, NKT, QO], F16, kind="ExternalInput")
    wk_h = nc.dram_tensor("wk", [128, NKT, KO], F16, kind="ExternalInput")
    wv_h = nc.dram_tensor("wv", [128, NKT, KO], F16, kind="ExternalInput")
    wo_h = nc.dram_tensor("wo", [128, 4, D], F16, kind="ExternalInput")
    bq_h = nc.dram_tensor("bq2", [128, 4], F32, kind="ExternalInput")
    bk_h = nc.dram_tensor("bk2", [128, 1], F32, kind="ExternalInput")
    bv_h = nc.dram_tensor("bv2", [128, 1], F32, kind="ExternalInput")
    out_h = nc.dram_tensor("out", [S, D], F16, kind="ExternalOutput")

    with tile.TileContext(nc) as tc:
        _emit(nc, tc, xT_h, wq_h, wk_h, wv_h, wo_h, bq_h, bk_h, bv_h, out_h)
    nc.compile()
    return nc


def _emit(nc, tc, xT_h, wq_h, wk_h, wv_h, wo_h, bq_h, bk_h, bv_h, out_h):
    from contextlib import ExitStack

    ctx = ExitStack()
    with ctx:
        consts = ctx.enter_context(tc.tile_pool(name="consts", bufs=1))
        persist = ctx.enter_context(tc.tile_pool(name="persist", bufs=1))
        xtp = ctx.enter_context(tc.tile_pool(name="xt", bufs=4))
        ep = ctx.enter_context(tc.tile_pool(name="expst", bufs=12))
        vnp = ctx.enter_context(tc.tile_pool(name="vnat", bufs=4))
        rawp = ctx.enter_context(tc.tile_pool(name="ctxraw", bufs=2))
        rdbp = ctx.enter_context(tc.tile_pool(name="rdb", bufs=2))
        obp = ctx.enter_context(tc.tile_pool(name="osb", bufs=4))
        spps = ctx.enter_context(tc.tile_pool(name="spps", bufs=2, space="PSUM"))
        ctxps = ctx.enter_context(tc.tile_pool(name="ctxps", bufs=1, space="PSUM"))
        # one-shot PSUM users (ph3 chunks): released after one eviction
        shotps = ctx.enter_context(tc.tile_pool(name="shotps", bufs=1, space="PSUM"))
        # multi-slot accumulations (K/V/Q proj), held across interleaved
        # aux items -- must not share a pool with one-shot users
        accps = ctx.enter_context(tc.tile_pool(name="accps", bufs=1, space="PSUM"))

        # persistent SBUF tensors
        qt_sb = persist.tile([128, 4, S], F16)      # Q^T [dim-in-pair, pair, token]
        ktd_sb = persist.tile([128, KVL, S], F16)   # K^T, kv dims dup both halves
        vt_sb = persist.tile([128, S], F16)         # V^T [kv dims (2x64), token]
        vaug_sb = persist.tile([128, NTT, KVL, 128], F16)  # [tok, tt, kv, dim|ones]
        ctxT_sb = persist.tile([128, 4, S], F16)    # [dim-in-pair, pair, token]

        wq_sb = persist.tile([128, NKT, QO], F16)
        wk_sb = persist.tile([128, NKT, KO], F16)
        wv_sb = persist.tile([128, NKT, KO], F16)
        wo_sb = persist.tile([128, 4, D], F16)

        bq_sb = consts.tile([128, 4], F32)
        bk_sb = consts.tile([128, 1], F32)
        bv_sb = consts.tile([128, 1], F32)

        # ---- input DMAs, ordered by first use ----
        nc.sync.dma_start(out=bq_sb, in_=bq_h[:])
        nc.sync.dma_start(out=bk_sb, in_=bk_h[:])
        nc.sync.dma_start(out=bv_sb, in_=bv_h[:])
        nc.sync.dma_start(out=wk_sb, in_=wk_h[:])
        nc.sync.dma_start(out=wv_sb, in_=wv_h[:])

        xts = []
        for nt in range(4):
            xts.append(xtp.tile([128, NKT, 512], F16, tag="xt", name=f"xt{nt}"))
        nc.sync.dma_start(out=xts[0], in_=xT_h[:, 0, :, :])
        # pair-0 query weights first so attention can start early
        nc.sync.dma_start(out=wq_sb[:, :, 0:128], in_=wq_h[:, :, 0:128])
        nc.sync.dma_start(out=xts[1], in_=xT_h[:, 1, :, :])
        nc.sync.dma_start(out=xts[2], in_=xT_h[:, 2, :, :])
        nc.sync.dma_start(out=xts[3], in_=xT_h[:, 3, :, :])
        nc.sync.dma_start(out=wq_sb[:, :, 128:512], in_=wq_h[:, :, 128:512])
        nc.sync.dma_start(out=wo_sb, in_=wo_h[:])

        ones_init = [False]

        # ---------- aux work item generators (each returns a closure) ----------
        kproj_state = {}

        def kproj_piece(nt, piece):
            def run():
                ns = slice(nt * 512, (nt + 1) * 512)
                if piece == 0:
                    kproj_state[nt] = accps.tile([128, 512], F32, tag="acc", name=f"kp{nt}")
                ps = kproj_state[nt]
                for kt in range(piece * 4, piece * 4 + 4):
                    nc.tensor.matmul(
                        ps[:, :], lhsT=wk_sb[:, kt, :], rhs=xts[nt][:, kt, :],
                        start=(kt == 0), stop=(kt == NKT - 1))
                if piece == 3:
                    # bias + cast + duplicate kv dims into both partition halves
                    for kv in range(KVL):
                        src = ps[kv * 64:(kv + 1) * 64, 0:512]
                        sc = bk_sb[kv * 64:(kv + 1) * 64, 0:1]
                        nc.vector.tensor_scalar_add(ktd_sb[0:64, kv, ns], src, sc)
                        nc.vector.tensor_scalar_add(ktd_sb[64:128, kv, ns], src, sc)
            return run

        vproj_state = {}
        vnat_state = {}

        def vproj_piece(nt, piece):
            def run():
                ns = slice(nt * 512, (nt + 1) * 512)
                if piece == 0:
                    vproj_state[nt] = accps.tile([128, 512], F32, tag="acc", name=f"vp{nt}")
                ps = vproj_state[nt]
                for kt in range(piece * 4, piece * 4 + 4):
                    nc.tensor.matmul(
                        ps[:, :], lhsT=wv_sb[:, kt, :], rhs=xts[nt][:, kt, :],
                        start=(kt == 0), stop=(kt == NKT - 1))
                if piece == 3:
                    nc.vector.tensor_scalar_add(vt_sb[:, ns], ps[:, :],
                                                bv_sb[:, 0:1])
                    # V -> natural [token, dim] layout via DMA xbar transpose
                    for tt in range(nt * 4, nt * 4 + 4):
                        vn = vnp.tile([128, 128], F16, tag="vn", name=f"vn{tt}")
                        vnat_state[tt] = vn
                        nc.sync.dma_start_transpose(
                            vn, vt_sb[:, tt * 128:(tt + 1) * 128])
            return run

        def transp_copies(tt):
            # DVE-only: vnat slices into vaug (ones half pre-set by memset)
            def run():
                if not ones_init[0]:
                    nc.vector.memset(vaug_sb, 1.0)
                    ones_init[0] = True
                vn = vnat_state[tt]
                for kv in range(KVL):
                    nc.vector.tensor_copy(
                        vaug_sb[:, tt, kv, 0:64],
                        vn[:, kv * 64:(kv + 1) * 64])
            return run

        qproj_state = {}

        def qproj_piece(qt, mt, piece):
            def run():
                ns = slice(qt * 512, (qt + 1) * 512)
                if piece == 0:
                    qproj_state[(qt, mt)] = accps.tile([128, 512], F32, tag="acc", name=f"qp{qt}_{mt}")
                ps = qproj_state[(qt, mt)]
                for kt in range(piece * 4, piece * 4 + 4):
                    nc.tensor.matmul(
                        ps[:, :],
                        lhsT=wq_sb[:, kt, mt * 128:(mt + 1) * 128],
                        rhs=xts[qt][:, kt, :],
                        start=(kt == 0), stop=(kt == NKT - 1))
                if piece == 3:
                    nc.vector.tensor_scalar_add(
                        qt_sb[:, mt, ns], ps[:, :], bq_sb[:, mt:mt + 1])
            return run

        def ph3_chunk(qt, tt, dn, alt=False):
            def run():
                ts_ = slice(tt * 128, (tt + 1) * 128)
                ds_ = slice(dn * 512, (dn + 1) * 512)
                if alt:
                    ps = accps.tile([128, 512], F32, tag="acc", name=f"p3a{tt}_{dn}")
                else:
                    ps = shotps.tile([128, 512], F32, tag="shot", name=f"p3s{tt}_{dn}")
                for c in range(4):
                    nc.tensor.matmul(
                        ps[:, :],
                        lhsT=ctxT_sb[:, c, ts_],
                        rhs=wo_sb[:, c, ds_],
                        start=(c == 0), stop=(c == 3),
                    )
                ob = obp.tile([128, 512], F16)
                nc.scalar.copy(ob, ps)
                nc.sync.dma_start(out=out_h[ts_, ds_], in_=ob)
            return run

        # ---------- pre-attention head ----------
        for piece in range(4):
            kproj_piece(0, piece)()
        for piece in range(4):
            qproj_piece(0, 0, piece)()
        for piece in range(4):
            vproj_piece(0, piece)()
        for tt in range(4):
            transp_copies(tt)()

        # ---------- aux slot plans (per qt: NSLOT slots) ----------
        def grp(f, *idx_lists):
            return [f(*i) for i in idx_lists]

        slots = {qt: [[] for _ in range(NSLOT)] for qt in range(NQT)}
        s0 = slots[0]
        s0[0] = [kproj_piece(1, p) for p in range(4)]
        s0[1] = [vproj_piece(1, p) for p in range(4)] + \
                [transp_copies(t) for t in range(4, 8)]
        s0[2] = [kproj_piece(2, p) for p in range(4)]
        s0[3] = [vproj_piece(2, p) for p in range(4)] + \
                [transp_copies(t) for t in range(8, 12)]
        s0[4] = [kproj_piece(3, p) for p in range(4)]
        s0[5] = [vproj_piece(3, p) for p in range(4)] + \
                [transp_copies(t) for t in range(12, 16)]
        s0[6] = [qproj_piece(0, 1, p) for p in range(4)]
        s0[8] = [qproj_piece(0, 2, p) for p in range(4)]
        s0[10] = [qproj_piece(0, 3, p) for p in range(4)]
        s0[12] = [qproj_piece(1, 0, p) for p in range(2)]
        s0[13] = [qproj_piece(1, 0, p) for p in range(2, 4)]
        s0[15] = [qproj_piece(1, 1, p) for p in range(2)]
        s0[16] = [qproj_piece(1, 1, p) for p in range(2, 4)]
        s0[18] = [qproj_piece(1, 2, p) for p in range(2)]
        s0[19] = [qproj_piece(1, 2, p) for p in range(2, 4)]
        s0[21] = [qproj_piece(1, 3, p) for p in range(2)]
        s0[22] = [qproj_piece(1, 3, p) for p in range(2, 4)]

        for qt in (1, 2):
            sq = slots[qt]
            # output projection of qt-1: one chunk per slot
            items = [(tt, dn) for tt in range((qt - 1) * 4, (qt - 1) * 4 + 4)
                     for dn in range(4)]
            for k, (tt, dn) in enumerate(items):
                sq[k] = [ph3_chunk(qt - 1, tt, dn)]
            # Q projection of qt+1: 2 pieces per slot
            base = 17
            for mt in range(4):
                sq[base + 3 * mt] = [qproj_piece(qt + 1, mt, p) for p in range(2)]
                sq[base + 3 * mt + 1] = [qproj_piece(qt + 1, mt, p)
                                         for p in range(2, 4)]
        sq = slots[3]
        items = [(tt, dn) for tt in range(8, 12) for dn in range(4)]
        for k, (tt, dn) in enumerate(items):
            sq[k] = [ph3_chunk(2, tt, dn)]

        # ---------- main attention loop ----------
        for qt in range(NQT):
            qs = slice(qt * 512, (qt + 1) * 512)
            for pair in range(4):
                kv = pair // 2
                ctx_t = ctxps.tile([128, 2, 512], F32, tag="ctx")
                for g in range(8):
                    sp = [spps.tile([128, 2, 512], F32, tag="sp", name=f"sp{i}")
                          for i in range(2)]
                    for j in range(2):
                        kt = g * 2 + j
                        ks = slice(kt * 128, (kt + 1) * 128)
                        for i in range(2):
                            nc.tensor.matmul(
                                sp[i][:, j, :],
                                lhsT=ktd_sb[i * 64:(i + 1) * 64, kv, ks],
                                rhs=qt_sb[i * 64:(i + 1) * 64, pair, qs],
                                start=True, stop=True,
                                tile_position=(i * 64, 0),
                            )
                    es = []
                    for i in range(2):
                        e = ep.tile([128, 2, 512], F16)
                        nc.scalar.activation(
                            e[:, :, :], sp[i][:, :, :],
                            mybir.ActivationFunctionType.Exp, scale=0.125)
                        es.append(e)
                    for j in range(2):
                        kt = g * 2 + j
                        for i in range(2):
                            nc.tensor.matmul(
                                ctx_t[:, i, :],
                                lhsT=vaug_sb[:, kt, kv, :],
                                rhs=es[i][:, j, :],
                                start=(kt == 0), stop=(kt == NTT - 1),
                            )
                    for item in slots[qt][pair * 8 + g]:
                        item()

                # evict ctx bank fast, then normalize off the raw copy
                raw = rawp.tile([128, 2, 512], F32, tag="raw")
                nc.vector.tensor_copy(raw, ctx_t)
                rdb = rdbp.tile([64, 2, 512], F32, tag="rdb")
                nc.vector.reciprocal(rdb, raw[64:128, :, :])
                for i in range(2):
                    nc.vector.tensor_tensor(
                        ctxT_sb[i * 64:(i + 1) * 64, pair, qs],
                        raw[0:64, i, :],
                        rdb[:, i, :],
                        mybir.AluOpType.mult,
                    )

        # ---------- tail: output projection of qt3 (alternate psum pools) ----
        tail = [(tt, dn) for tt in range(12, 16) for dn in range(4)]
        for n, (tt, dn) in enumerate(tail):
            ph3_chunk(3, tt, dn, alt=(n % 2 == 1))()


def _get_nc():
    if "nc" not in _CACHE:
        _CACHE["nc"] = _build_nc()
    return _CACHE["nc"]


def kernel(x, Wq, bq, Wk, bk, Wv, bv, Wo, bo, _trace=False):
    x = np.asarray(x, np.float32)
    Wq = np.asarray(Wq, np.float32)
    bq = np.asarray(bq, np.float32)
    Wk = np.asarray(Wk, np.float32)
    bk = np.asarray(bk, np.float32)
    Wv = np.asarray(Wv, np.float32)
    bv = np.asarray(bv, np.float32)
    Wo = np.asarray(Wo, np.float32)
    bo = np.asarray(bo, np.float32)

    nc = _get_nc()
    in_maps = []
    for r in range(8):
        b, g = divmod(r, 4)
        qsl = slice(g * 512, (g + 1) * 512)
        ksl = slice(g * 128, (g + 1) * 128)
        xb = x[b].T.astype(np.float16).reshape(16, 128, 4, 512)
        in_maps.append({
            "xT": np.ascontiguousarray(xb.transpose(1, 2, 0, 3)),
            "wq": np.ascontiguousarray(
                Wq[:, qsl].astype(np.float16).reshape(16, 128, 512)
                .transpose(1, 0, 2)),
            "wk": np.ascontiguousarray(
                Wk[:, ksl].astype(np.float16).reshape(16, 128, 128)
                .transpose(1, 0, 2)),
            "wv": np.ascontiguousarray(
                Wv[:, ksl].astype(np.float16).reshape(16, 128, 128)
                .transpose(1, 0, 2)),
            "wo": np.ascontiguousarray(
                Wo[qsl, :].astype(np.float16).reshape(4, 128, 2048)
                .transpose(1, 0, 2)),
            "bq2": np.ascontiguousarray(bq[qsl].reshape(4, 128).T),
            "bk2": np.ascontiguousarray(bk[ksl].reshape(128, 1)),
            "bv2": np.ascontiguousarray(bv[ksl].reshape(128, 1)),
        })

    res = run_bass_kernel_spmd(nc, in_maps, list(range(8)), trace=_trace)
    out = np.zeros((2, S, D), np.float64)
    for r in range(8):
        out[r // 4] += res.results[r]["out"].astype(np.float64)
    out += bo.astype(np.float64)
    result = out.astype(np.float32)
    if _trace:
        return result, res
    return result
